# revision 18
# baseline (speedup 1.0000x reference)
"""Trainium2 Bass kernel for causal self-attention (B=2, S=2048, D=1024, H=16).

Sharding: 8 cores = 2 (batch) x 4 (head groups of 4 heads) — data parallel on
batch, tensor parallel on heads. Each core computes, for its batch b and its
4 heads (256 of the 1024 model dims):

  qT/kT = Wq_slice^T x^T            transposed layouts [head_dim, seq], fp16
  v     = x Wv_slice                natural layout [seq, head_dim], fp16
  per head pair (2 heads share the 128 partitions):
    scoresT[kv, q] blocks on PE (two row-packed K=64 matmuls, concurrent),
    exp on ACT (psum -> fp16 sbuf), causal mask multiply on DVE (fp16 2x),
    P^T V + replicated ones-row denominators on PE (col-packed M=64),
    normalize: cross-quadrant reciprocal_approx_fast + one tensor_mul.
  oT_partial = Wo_slice^T attnT     [1024, seq] fp16 partial

Host: feeds x^T and fp16 weight slices, sums the 4 partials per batch
(the "all-reduce" of the o-projection), transposes, adds bo.

Schedule notes (v2): input DMAs are issued in consumption order across three
engine queues (sync: xt in 3 column groups x 8 k-tiles; vector: wq/wk early;
gpsimd: wv/wo/msk/biases) so the first projection starts ~1.5us in. The
o-projection PSUM->SBUF copies alternate between gpsimd and vector and write
fp16 (half the output DMA). Projection/o-proj fillers are interleaved one per
attention kv-iteration, with o-proj deferred to the late attention calls so
the PE has work while ACT drains the last exp tiles.
"""

import numpy as np

import concourse.bacc as bacc
import concourse.tile as tile
from concourse import mybir
from concourse.bass_utils import run_bass_kernel_spmd

B, S, D, H = 2, 2048, 1024, 16
HD = D // H          # 64
P = 128
NCORES = 8
GROUPS = 4           # head groups (tensor parallel)
HPG = H // GROUPS    # 4 heads per group
CD = HPG * HD        # 256 local head dims per core
QT = 512             # q tile (matmul free dim)
KT = 128             # kv tile (psum partition dim)
NQT = S // QT        # 4
NKT = S // KT        # 16
KD = D // P          # 8 contraction tiles over the model dim

F32 = mybir.dt.float32
F16 = mybir.dt.float16
EXP = mybir.ActivationFunctionType.Exp

_NC_CACHE = {}


def _build_nc():
    if "nc" in _NC_CACHE:
        return _NC_CACHE["nc"]
    nc = bacc.Bacc()
    xt = nc.declare_dram_parameter("xt", [D, S], F16, isOutput=False)
    wq = nc.declare_dram_parameter("wq", [D, CD], F16, isOutput=False)
    wk = nc.declare_dram_parameter("wk", [D, CD], F16, isOutput=False)
    wv = nc.declare_dram_parameter("wv", [D, CD], F16, isOutput=False)
    wo = nc.declare_dram_parameter("wo", [CD, D], F16, isOutput=False)
    bq = nc.declare_dram_parameter("bq", [CD], F32, isOutput=False)
    bk = nc.declare_dram_parameter("bk", [CD], F32, isOutput=False)
    bv = nc.declare_dram_parameter("bv", [HPG, HD], F32, isOutput=False)
    msk = nc.declare_dram_parameter("msk", [4, P, QT], F16, isOutput=False)
    ot = nc.declare_dram_parameter("ot", [D, S], F16, isOutput=True)

    import concourse.bass as bass

    with tile.TileContext(nc) as tc:
        with tc.tile_pool(name="consts", bufs=1) as consts, \
             tc.tile_pool(name="work", bufs=3) as work, \
             tc.tile_pool(name="ps_s", bufs=2, space="PSUM") as ps_s, \
             tc.tile_pool(name="ps_av", bufs=2, space="PSUM") as ps_av, \
             tc.tile_pool(name="ps_po", bufs=2, space="PSUM") as ps_po:

            # ---- constant / persistent SBUF tensors ----
            xt_sb = consts.tile([P, KD, S], F16)
            wq_sb = consts.tile([P, KD, CD], F16)
            wk_sb = consts.tile([P, KD, CD], F16)
            wv_sb = consts.tile([P, KD, CD], F16)
            wo_sb = consts.tile([P, 2, D], F16)
            bq_sb = consts.tile([P, 2], F32)
            bk_sb = consts.tile([P, 2], F32)
            bv_sb = consts.tile([P, HPG, HD], F32)
            msk_sb = consts.tile([P, 4, QT], F16)
            qT_sb = consts.tile([P, 2, S], F16)
            kT_sb = consts.tile([P, 2, S], F16)
            v2_sb = consts.tile([P, NKT, 2, 3 * HD], F16)
            aT_sb = consts.tile([P, 2, NQT, QT], F16)
            o3g0_sb = consts.tile([P, D // P, QT], F16)

            # ---- input DMAs in consumption order, spread across queues ----
            # sync queue: xt in 3 column groups (cols 0:512 first so the
            # t=0 projections can start), each split by k-tile.
            xt_r = xt[:, :].rearrange("(k p) s -> p k s", p=P)
            wq_r = wq[:, :].rearrange("(k p) c -> p k c", p=P)
            wk_r = wk[:, :].rearrange("(k p) c -> p k c", p=P)
            wv_r = wv[:, :].rearrange("(k p) c -> p k c", p=P)
            # scalar queue: wq then wk (small, needed first; ACT is idle
            # until the first exp)
            nc.scalar.dma_start(out=wq_sb, in_=wq_r)
            nc.scalar.dma_start(out=wk_sb, in_=wk_r)
            for kt in range(KD):
                nc.sync.dma_start(out=xt_sb[:, kt, 0:2 * QT],
                                  in_=xt_r[:, kt, 0:2 * QT])
            # gpsimd queue: wv, masks, biases, wo
            nc.gpsimd.dma_start(out=wv_sb, in_=wv_r)
            nc.gpsimd.dma_start(
                out=msk_sb, in_=msk[:, :, :].rearrange("r p c -> p r c"))
            nc.gpsimd.dma_start(out=bq_sb,
                                in_=bq[:].rearrange("(m p) -> p m", p=P))
            nc.gpsimd.dma_start(out=bk_sb,
                                in_=bk[:].rearrange("(m p) -> p m", p=P))
            bv_ap = bv[:, :]
            bv_bc = bass.AP(tensor=bv_ap.tensor, offset=bv_ap.offset,
                            ap=[[0, P]] + list(bv_ap.ap))
            nc.gpsimd.dma_start(out=bv_sb, in_=bv_bc)
            nc.gpsimd.dma_start(out=wo_sb,
                                in_=wo[:, :].rearrange("(g p) e -> p g e", p=P))
            for kt in range(KD):
                nc.sync.dma_start(out=xt_sb[:, kt, 2 * QT:],
                                  in_=xt_r[:, kt, 2 * QT:])
            nc.vector.memset(v2_sb[:, :, :, HD:2 * HD], 1.0)

            # ---- helpers ----
            def proj_qk(w_sb, b_sb, dst, mt, nts):
                pss = [ps_po.tile([P, QT], F32, tag="po", name=f"ps_qk{j}")
                       for j in range(len(nts))]
                for kt in range(KD):
                    lhs = w_sb[:, kt, mt * P:(mt + 1) * P]
                    for j, nt in enumerate(nts):
                        nc.tensor.matmul(
                            pss[j], lhs,
                            xt_sb[:, kt, nt * QT:(nt + 1) * QT],
                            start=(kt == 0), stop=(kt == KD - 1))
                for j, nt in enumerate(nts):
                    nc.vector.tensor_scalar_add(
                        dst[:, mt, nt * QT:(nt + 1) * QT], pss[j],
                        b_sb[:, mt:mt + 1])

            def proj_v(jt0, jt1):
                for jt in range(jt0, jt1):
                    ps = ps_po.tile([P, QT], F32, tag="po", name="ps_v")
                    for kt in range(KD):
                        nc.tensor.matmul(
                            ps[:, :CD], xt_sb[:, kt, jt * P:(jt + 1) * P],
                            wv_sb[:, kt, :],
                            start=(kt == 0), stop=(kt == KD - 1))
                    psh = ps[:, :CD].rearrange("p (h d) -> p h d", h=HPG)
                    # even heads -> cols 0:64, odd heads -> cols 128:192
                    nc.vector.tensor_add(
                        v2_sb[:, jt, :, 0:HD], psh[:, 0::2, :], bv_sb[:, 0::2, :])
                    nc.vector.tensor_add(
                        v2_sb[:, jt, :, 2 * HD:3 * HD], psh[:, 1::2, :],
                        bv_sb[:, 1::2, :])

            def attention(t, g, fillers=()):
                n_kv = 4 * (t + 1)
                # bank A: rows 0:64 = attn h(2g), rows 64:128 = denom h(2g)
                # bank B: rows 0:64 = denom h(2g+1), rows 64:128 = attn h(2g+1)
                av_a = ps_av.tile([P, QT], F32, tag="avden", name="av_a")
                av_b = ps_av.tile([P, QT], F32, tag="avden", name="av_b")
                for kv in range(n_kv):
                    if kv < len(fillers) and fillers[kv] is not None:
                        fillers[kv]()
                    r = kv - 4 * t
                    v0 = KT * r if r >= 1 else 0    # first valid q col
                    s = ps_s.tile([P, 2 * QT], F32, tag="s", name="s")
                    for idx in range(2):
                        p0 = 64 * idx
                        nc.tensor.matmul(
                            s[:, idx * QT + v0:(idx + 1) * QT],
                            kT_sb[p0:p0 + 64, g, kv * KT:(kv + 1) * KT],
                            qT_sb[p0:p0 + 64, g, t * QT + v0:(t + 1) * QT],
                            start=True, stop=True)
                    p_t = work.tile([P, 2 * QT], F16, tag="pt", name="p_t")

                    def two_range(tile_ap, off):
                        # AP covering [off:QT] and [QT+off:2QT] in one pattern
                        base = tile_ap[:, :]
                        ap = [list(base.ap[0]), [QT, 2], [1, QT - off]]
                        return bass.AP(tensor=base.tensor,
                                       offset=base.offset + off, ap=ap)

                    if r < 1:
                        nc.scalar.activation(p_t, s, EXP)
                    else:
                        nc.scalar.activation(two_range(p_t, v0),
                                             two_range(s, v0), EXP)
                    if r >= 0:
                        mbase = msk_sb[:, r, :]
                        m2 = bass.AP(tensor=mbase.tensor,
                                     offset=mbase.offset + v0,
                                     ap=[list(mbase.ap[0]), [0, 2],
                                         [1, QT - v0]])
                        nc.vector.tensor_mul(two_range(p_t, v0),
                                             two_range(p_t, v0), m2)
                    for idx, bank in ((0, av_a), (1, av_b)):
                        rhs = p_t[:, idx * QT + v0:(idx + 1) * QT]
                        lhsT = v2_sb[:, kv, g, HD * idx:HD * idx + 2 * HD]
                        nc.tensor.matmul(
                            bank[:, v0:], lhsT, rhs,
                            start=(kv == 0), stop=(kv == n_kv - 1))
                # normalize: aT = av / den, denominators shifted across
                # partition halves via a small SBUF->SBUF DMA. NOTE: DVE ops
                # at non-zero base partitions mis-route on HW, so the
                # reciprocals run full-width from partition 0.
                rca = work.tile([P, QT], F32, tag="rca", name="rca")
                rcb = work.tile([P, QT], F32, tag="rcb", name="rcb")
                rc2 = work.tile([P, QT], F32, tag="rc2", name="rc2")
                nc.vector.reciprocal_approx_fast(rca, av_a)
                nc.vector.reciprocal_approx_fast(rcb, av_b)
                nc.sync.dma_start(out=rc2[0:64, :], in_=rca[64:128, :])
                nc.sync.dma_start(out=rc2[64:128, :], in_=rcb[0:64, :])
                nc.vector.tensor_mul(aT_sb[0:64, g, t, :], av_a[0:64, :],
                                     rc2[0:64, :])
                nc.vector.tensor_mul(aT_sb[64:128, g, t, :], av_b[64:128, :],
                                     rc2[64:128, :])

            def oproj(t, mts=None):
                for mt_e in (range(D // P) if mts is None else mts):
                    ps = ps_po.tile([P, QT], F32, tag="po", name="ps_o")
                    for g in range(2):
                        nc.tensor.matmul(
                            ps, wo_sb[:, g, mt_e * P:(mt_e + 1) * P],
                            aT_sb[:, g, t, :],
                            start=(g == 0), stop=(g == 1))
                    ot_t = work.tile([P, QT], F16, tag="ot", name="ot_t")
                    nc.vector.tensor_copy(ot_t, ps)
                    nc.sync.dma_start(
                        out=ot[mt_e * P:(mt_e + 1) * P, t * QT:(t + 1) * QT],
                        in_=ot_t)

            def oproj3_g0(mt_e):
                # g=0 half of the t=3 o-projection, stashed in SBUF fp16;
                # runs as att(3,1) filler so the PE/DVE stay busy while ACT
                # drains the last exp tiles
                ps = ps_po.tile([P, QT], F32, tag="po", name="ps_o3a")
                nc.tensor.matmul(ps, wo_sb[:, 0, mt_e * P:(mt_e + 1) * P],
                                 aT_sb[:, 0, 3, :], start=True, stop=True)
                nc.vector.tensor_copy(o3g0_sb[:, mt_e, :], ps)

            def oproj3_g1():
                # tail: g=1 matmul + add the stashed g0 half; copies and
                # output DMAs alternate vector/sync with scalar (ACT idle)
                for mt_e in range(D // P):
                    ps = ps_po.tile([P, QT], F32, tag="po", name="ps_o3b")
                    nc.tensor.matmul(ps, wo_sb[:, 1, mt_e * P:(mt_e + 1) * P],
                                     aT_sb[:, 1, 3, :], start=True, stop=True)
                    ot_t = work.tile([P, QT], F16, tag="ot", name="ot_t")
                    nc.vector.tensor_add(ot_t, ps, o3g0_sb[:, mt_e, :])
                    deng = nc.sync if mt_e % 2 == 0 else nc.scalar
                    deng.dma_start(
                        out=ot[mt_e * P:(mt_e + 1) * P, 3 * QT:4 * QT],
                        in_=ot_t)

            # ---- filler-interleaved schedule: the PE stream alternates
            # one projection/o-proj chunk per attention kv-iteration; o-proj
            # chunks are deferred to the late attention calls so the tail
            # still has PE work while ACT drains ----
            def fq(mt, nt):
                return lambda: proj_qk(wq_sb, bq_sb, qT_sb, mt, [nt])

            def fk(mt, nt):
                return lambda: proj_qk(wk_sb, bk_sb, kT_sb, mt, [nt])

            def fv(jt):
                return lambda: proj_v(jt, jt + 1)

            def fo(t, m0):
                return lambda: oproj(t, mts=[m0, m0 + 1])

            def fq2(mt, n0, n1):
                return lambda: proj_qk(wq_sb, bq_sb, qT_sb, mt, [n0, n1])

            def fk2(mt, n0, n1):
                return lambda: proj_qk(wk_sb, bk_sb, kT_sb, mt, [n0, n1])

            # prefix: just enough for att(0,0)
            proj_qk(wq_sb, bq_sb, qT_sb, 0, [0])
            proj_qk(wk_sb, bk_sb, kT_sb, 0, [0])
            proj_v(0, 1)
            attention(0, 0, [fv(1), fv(2), fv(3), fq(1, 0)])
            attention(0, 1, [fk(1, 0), fq(0, 1), fk(0, 1), fq(1, 1)])
            attention(1, 0, [fk(1, 1), fq2(0, 2, 3), None, fk2(0, 2, 3),
                             fv(4), fv(5), fv(6), fv(7)])
            attention(1, 1, [fq2(1, 2, 3), None, fk2(1, 2, 3), None,
                             fv(8), fv(9), fv(10), fv(11)])
            attention(2, 0, [fo(0, 0), None, fo(0, 2), None,
                             fo(0, 4), None, fo(0, 6), None,
                             fv(12), fv(13), fv(14), fv(15)])
            attention(2, 1, [fo(1, 0), None, fo(1, 2), None,
                             fo(1, 4), None, fo(1, 6), None,
                             None, None, None, None])
            attention(3, 0, [fo(2, 0), None, fo(2, 2), None,
                             fo(2, 4), None, fo(2, 6), None,
                             None, None, None, None, None, None, None, None])
            attention(3, 1, [lambda: oproj3_g0(0), None, lambda: oproj3_g0(1),
                             None, lambda: oproj3_g0(2), None,
                             lambda: oproj3_g0(3), None, lambda: oproj3_g0(4),
                             None, lambda: oproj3_g0(5), None,
                             lambda: oproj3_g0(6), None, lambda: oproj3_g0(7),
                             None])
            oproj3_g1()

    nc.compile()
    _NC_CACHE["nc"] = nc
    return nc


def _make_masks():
    # msk[r, p, c] for the 4 diagonal kv offsets r: valid iff p <= c - 128 r
    m = np.zeros((4, P, QT), dtype=np.float16)
    pp = np.arange(P)[:, None]
    cc = np.arange(QT)[None, :]
    for r in range(4):
        m[r] = (pp <= cc - KT * r).astype(np.float16)
    return m


def _in_maps(x, Wq, bq, Wk, bk, Wv, bv, Wo):
    scale = np.float32(1.0 / np.sqrt(HD))
    masks = _make_masks()
    maps = []
    for core in range(NCORES):
        b, g = divmod(core, GROUPS)
        csl = slice(g * CD, (g + 1) * CD)
        maps.append({
            "xt": np.ascontiguousarray(x[b].T).astype(np.float16),
            "wq": np.ascontiguousarray(Wq[:, csl] * scale).astype(np.float16),
            "wk": np.ascontiguousarray(Wk[:, csl]).astype(np.float16),
            "wv": np.ascontiguousarray(Wv[:, csl]).astype(np.float16),
            "wo": np.ascontiguousarray(Wo[csl, :]).astype(np.float16),
            "bq": np.ascontiguousarray(bq[csl] * scale).astype(np.float32),
            "bk": np.ascontiguousarray(bk[csl]).astype(np.float32),
            "bv": np.ascontiguousarray(bv[csl]).reshape(HPG, HD).astype(np.float32),
            "msk": masks,
        })
    return maps


def kernel_with_results(x, Wq, bq, Wk, bk, Wv, bv, Wo, bo, trace=False):
    nc = _build_nc()
    maps = _in_maps(x, Wq, bq, Wk, bk, Wv, bv, Wo)
    kwargs = {}
    if trace:
        kwargs = dict(trace=True, trace_cores=[0])
    res = run_bass_kernel_spmd(nc, maps, core_ids=list(range(NCORES)), **kwargs)
    out = np.zeros((B, S, D), dtype=np.float32)
    for b in range(B):
        acc = np.zeros((D, S), dtype=np.float32)
        for g in range(GROUPS):
            acc += res.results[b * GROUPS + g]["ot"].astype(np.float32)
        out[b] = acc.T + np.asarray(bo, dtype=np.float32)[None, :]
    return out, res


def kernel(x, Wq, bq, Wk, bk, Wv, bv, Wo, bo):
    out, _ = kernel_with_results(x, Wq, bq, Wk, bk, Wv, bv, Wo, bo, trace=False)
    return out


# revision 21
# speedup vs baseline: 1.1716x; 1.1716x over previous
"""Trainium2 Bass kernel for causal self-attention (B=2, S=2048, D=1024, H=16).

Sharding: 8 cores = 2 (batch) x 4 (head groups of 4 heads) — data parallel on
batch, tensor parallel on heads. Each core computes, for its batch b and its
4 heads (256 of the 1024 model dims):

  qT/kT = Wq_slice^T x^T            transposed layouts [head_dim, seq], fp16
  v     = x Wv_slice                natural layout [seq, head_dim], fp16
  per head pair (2 heads share the 128 partitions):
    scoresT[kv, q] blocks on PE (two row-packed K=64 matmuls, concurrent),
    exp on ACT (psum -> fp16 sbuf), causal mask multiply on DVE (fp16 2x),
    P^T V + replicated ones-row denominators on PE (col-packed M=64),
    normalize: cross-quadrant reciprocal_approx_fast + one tensor_mul.
  oT_partial = Wo_slice^T attnT     [1024, seq] fp16 partial

Host: feeds x^T and fp16 weight slices, sums the 4 partials per batch
(the "all-reduce" of the o-projection), transposes, adds bo.

Schedule notes (v2): input DMAs are issued in consumption order across three
engine queues (sync: xt in 3 column groups x 8 k-tiles; vector: wq/wk early;
gpsimd: wv/wo/msk/biases) so the first projection starts ~1.5us in. The
o-projection PSUM->SBUF copies alternate between gpsimd and vector and write
fp16 (half the output DMA). Projection/o-proj fillers are interleaved one per
attention kv-iteration, with o-proj deferred to the late attention calls so
the PE has work while ACT drains the last exp tiles.
"""

import numpy as np

import concourse.bacc as bacc
import concourse.tile as tile
from concourse import mybir
from concourse.bass_utils import run_bass_kernel_spmd

B, S, D, H = 2, 2048, 1024, 16
HD = D // H          # 64
P = 128
NCORES = 8
GROUPS = 4           # head groups (tensor parallel)
HPG = H // GROUPS    # 4 heads per group
CD = HPG * HD        # 256 local head dims per core
QT = 512             # q tile (matmul free dim)
KT = 128             # kv tile (psum partition dim)
NQT = S // QT        # 4
NKT = S // KT        # 16
KD = D // P          # 8 contraction tiles over the model dim

F32 = mybir.dt.float32
F16 = mybir.dt.float16
EXP = mybir.ActivationFunctionType.Exp

_NC_CACHE = {}


def _build_nc():
    if "nc" in _NC_CACHE:
        return _NC_CACHE["nc"]
    nc = bacc.Bacc()
    xt = nc.declare_dram_parameter("xt", [D, S], F16, isOutput=False)
    wq = nc.declare_dram_parameter("wq", [D, CD], F16, isOutput=False)
    wk = nc.declare_dram_parameter("wk", [D, CD], F16, isOutput=False)
    wv = nc.declare_dram_parameter("wv", [D, CD], F16, isOutput=False)
    wo = nc.declare_dram_parameter("wo", [CD, D], F16, isOutput=False)
    bq = nc.declare_dram_parameter("bq", [CD], F32, isOutput=False)
    bk = nc.declare_dram_parameter("bk", [CD], F32, isOutput=False)
    bv = nc.declare_dram_parameter("bv", [HPG, HD], F32, isOutput=False)
    msk = nc.declare_dram_parameter("msk", [4, P, QT], F16, isOutput=False)
    ot = nc.declare_dram_parameter("ot", [D, S], F16, isOutput=True)

    import concourse.bass as bass

    with tile.TileContext(nc) as tc:
        with tc.tile_pool(name="consts", bufs=1) as consts, \
             tc.tile_pool(name="work", bufs=3) as work, \
             tc.tile_pool(name="ps_s", bufs=2, space="PSUM") as ps_s, \
             tc.tile_pool(name="ps_av", bufs=2, space="PSUM") as ps_av, \
             tc.tile_pool(name="ps_po", bufs=2, space="PSUM") as ps_po:

            # ---- constant / persistent SBUF tensors ----
            xt_sb = consts.tile([P, KD, S], F16)
            wq_sb = consts.tile([P, KD, CD], F16)
            wk_sb = consts.tile([P, KD, CD], F16)
            wv_sb = consts.tile([P, KD, CD], F16)
            wo_sb = consts.tile([P, 2, D], F16)
            bq_sb = consts.tile([P, 2], F32)
            bk_sb = consts.tile([P, 2], F32)
            bv_sb = consts.tile([P, HPG, HD], F32)
            msk_sb = consts.tile([P, 4, QT], F16)
            qT_sb = consts.tile([P, 2, S], F16)
            kT_sb = consts.tile([P, 2, S], F16)
            v2_sb = consts.tile([P, NKT, 2, 3 * HD], F16)
            aT_sb = consts.tile([P, 2, NQT, QT], F16)

            # ---- input DMAs in consumption order, spread across queues ----
            # sync queue: xt in 3 column groups (cols 0:512 first so the
            # t=0 projections can start), each split by k-tile.
            xt_r = xt[:, :].rearrange("(k p) s -> p k s", p=P)
            wq_r = wq[:, :].rearrange("(k p) c -> p k c", p=P)
            wk_r = wk[:, :].rearrange("(k p) c -> p k c", p=P)
            wv_r = wv[:, :].rearrange("(k p) c -> p k c", p=P)
            # scalar queue: wq then wk (small, needed first; ACT is idle
            # until the first exp)
            nc.scalar.dma_start(out=wq_sb, in_=wq_r)
            nc.scalar.dma_start(out=wk_sb, in_=wk_r)
            for kt in range(KD):
                nc.sync.dma_start(out=xt_sb[:, kt, 0:2 * QT],
                                  in_=xt_r[:, kt, 0:2 * QT])
            # gpsimd queue: wv, masks, biases, wo
            nc.gpsimd.dma_start(out=wv_sb, in_=wv_r)
            nc.gpsimd.dma_start(
                out=msk_sb, in_=msk[:, :, :].rearrange("r p c -> p r c"))
            nc.gpsimd.dma_start(out=bq_sb,
                                in_=bq[:].rearrange("(m p) -> p m", p=P))
            nc.gpsimd.dma_start(out=bk_sb,
                                in_=bk[:].rearrange("(m p) -> p m", p=P))
            bv_ap = bv[:, :]
            bv_bc = bass.AP(tensor=bv_ap.tensor, offset=bv_ap.offset,
                            ap=[[0, P]] + list(bv_ap.ap))
            nc.gpsimd.dma_start(out=bv_sb, in_=bv_bc)
            nc.gpsimd.dma_start(out=wo_sb,
                                in_=wo[:, :].rearrange("(g p) e -> p g e", p=P))
            for kt in range(KD):
                nc.sync.dma_start(out=xt_sb[:, kt, 2 * QT:],
                                  in_=xt_r[:, kt, 2 * QT:])
            nc.vector.memset(v2_sb[:, :, :, HD:2 * HD], 1.0)

            # ---- helpers ----
            def proj_qk(w_sb, b_sb, dst, mt, nts):
                pss = [ps_po.tile([P, QT], F32, tag="po", name=f"ps_qk{j}")
                       for j in range(len(nts))]
                for kt in range(KD):
                    lhs = w_sb[:, kt, mt * P:(mt + 1) * P]
                    for j, nt in enumerate(nts):
                        nc.tensor.matmul(
                            pss[j], lhs,
                            xt_sb[:, kt, nt * QT:(nt + 1) * QT],
                            start=(kt == 0), stop=(kt == KD - 1))
                for j, nt in enumerate(nts):
                    nc.vector.tensor_scalar_add(
                        dst[:, mt, nt * QT:(nt + 1) * QT], pss[j],
                        b_sb[:, mt:mt + 1])

            def proj_v(jt0, jt1):
                for jt in range(jt0, jt1):
                    ps = ps_po.tile([P, QT], F32, tag="po", name="ps_v")
                    for kt in range(KD):
                        nc.tensor.matmul(
                            ps[:, :CD], xt_sb[:, kt, jt * P:(jt + 1) * P],
                            wv_sb[:, kt, :],
                            start=(kt == 0), stop=(kt == KD - 1))
                    psh = ps[:, :CD].rearrange("p (h d) -> p h d", h=HPG)
                    # even heads -> cols 0:64, odd heads -> cols 128:192
                    nc.vector.tensor_add(
                        v2_sb[:, jt, :, 0:HD], psh[:, 0::2, :], bv_sb[:, 0::2, :])
                    nc.vector.tensor_add(
                        v2_sb[:, jt, :, 2 * HD:3 * HD], psh[:, 1::2, :],
                        bv_sb[:, 1::2, :])

            def attention(t, g, fillers=()):
                n_kv = 4 * (t + 1)
                # bank A: rows 0:64 = attn h(2g), rows 64:128 = denom h(2g)
                # bank B: rows 0:64 = denom h(2g+1), rows 64:128 = attn h(2g+1)
                av_a = ps_av.tile([P, QT], F32, tag="avden", name="av_a")
                av_b = ps_av.tile([P, QT], F32, tag="avden", name="av_b")
                for kv in range(n_kv):
                    if kv < len(fillers) and fillers[kv] is not None:
                        fillers[kv]()
                    r = kv - 4 * t
                    v0 = KT * r if r >= 1 else 0    # first valid q col
                    s = ps_s.tile([P, 2 * QT], F32, tag="s", name="s")
                    for idx in range(2):
                        p0 = 64 * idx
                        nc.tensor.matmul(
                            s[:, idx * QT + v0:(idx + 1) * QT],
                            kT_sb[p0:p0 + 64, g, kv * KT:(kv + 1) * KT],
                            qT_sb[p0:p0 + 64, g, t * QT + v0:(t + 1) * QT],
                            start=True, stop=True)
                    p_t = work.tile([P, 2 * QT], F16, tag="pt", name="p_t")

                    def two_range(tile_ap, off):
                        # AP covering [off:QT] and [QT+off:2QT] in one pattern
                        base = tile_ap[:, :]
                        ap = [list(base.ap[0]), [QT, 2], [1, QT - off]]
                        return bass.AP(tensor=base.tensor,
                                       offset=base.offset + off, ap=ap)

                    if r < 1:
                        nc.scalar.activation(p_t, s, EXP)
                    else:
                        nc.scalar.activation(two_range(p_t, v0),
                                             two_range(s, v0), EXP)
                    if r >= 0:
                        mbase = msk_sb[:, r, :]
                        m2 = bass.AP(tensor=mbase.tensor,
                                     offset=mbase.offset + v0,
                                     ap=[list(mbase.ap[0]), [0, 2],
                                         [1, QT - v0]])
                        nc.vector.tensor_mul(two_range(p_t, v0),
                                             two_range(p_t, v0), m2)
                    for idx, bank in ((0, av_a), (1, av_b)):
                        rhs = p_t[:, idx * QT + v0:(idx + 1) * QT]
                        lhsT = v2_sb[:, kv, g, HD * idx:HD * idx + 2 * HD]
                        nc.tensor.matmul(
                            bank[:, v0:], lhsT, rhs,
                            start=(kv == 0), stop=(kv == n_kv - 1))
                # normalize: aT = av / den, denominators shifted across
                # partition halves via a small SBUF->SBUF DMA. NOTE: DVE ops
                # at non-zero base partitions mis-route on HW, so the
                # reciprocals run full-width from partition 0.
                rca = work.tile([P, QT], F32, tag="rca", name="rca")
                rcb = work.tile([P, QT], F32, tag="rcb", name="rcb")
                rc2 = work.tile([P, QT], F32, tag="rc2", name="rc2")
                nc.vector.reciprocal_approx_fast(rca, av_a)
                nc.vector.reciprocal_approx_fast(rcb, av_b)
                nc.sync.dma_start(out=rc2[0:64, :], in_=rca[64:128, :])
                nc.sync.dma_start(out=rc2[64:128, :], in_=rcb[0:64, :])
                nc.vector.tensor_mul(aT_sb[0:64, g, t, :], av_a[0:64, :],
                                     rc2[0:64, :])
                nc.vector.tensor_mul(aT_sb[64:128, g, t, :], av_b[64:128, :],
                                     rc2[64:128, :])

            def oproj(t, mts=None, tail=False):
                for mt_e in (range(D // P) if mts is None else mts):
                    ps = ps_po.tile([P, QT], F32, tag="po", name="ps_o")
                    for g in range(2):
                        nc.tensor.matmul(
                            ps, wo_sb[:, g, mt_e * P:(mt_e + 1) * P],
                            aT_sb[:, g, t, :],
                            start=(g == 0), stop=(g == 1))
                    ot_t = work.tile([P, QT], F16, tag="ot", name="ot_t")
                    # at the tail ACT is idle — split the PSUM->SBUF copies
                    # between vector and scalar to shorten the drain
                    if tail and mt_e % 2 == 1:
                        nc.scalar.copy(ot_t, ps)
                    else:
                        nc.vector.tensor_copy(ot_t, ps)
                    nc.sync.dma_start(
                        out=ot[mt_e * P:(mt_e + 1) * P, t * QT:(t + 1) * QT],
                        in_=ot_t)

            # ---- filler-interleaved schedule: the PE stream alternates
            # one projection/o-proj chunk per attention kv-iteration; o-proj
            # chunks are deferred to the late attention calls so the tail
            # still has PE work while ACT drains ----
            def fq(mt, nt):
                return lambda: proj_qk(wq_sb, bq_sb, qT_sb, mt, [nt])

            def fk(mt, nt):
                return lambda: proj_qk(wk_sb, bk_sb, kT_sb, mt, [nt])

            def fv(jt):
                return lambda: proj_v(jt, jt + 1)

            def fo(t, m0):
                return lambda: oproj(t, mts=[m0, m0 + 1])

            def fq2(mt, n0, n1):
                return lambda: proj_qk(wq_sb, bq_sb, qT_sb, mt, [n0, n1])

            def fk2(mt, n0, n1):
                return lambda: proj_qk(wk_sb, bk_sb, kT_sb, mt, [n0, n1])

            # prefix: just enough for att(0,0)
            proj_qk(wq_sb, bq_sb, qT_sb, 0, [0])
            proj_qk(wk_sb, bk_sb, kT_sb, 0, [0])
            proj_v(0, 1)
            attention(0, 0, [fv(1), fv(2), fv(3), fq(1, 0)])
            attention(0, 1, [fk(1, 0), fq(0, 1), fk(0, 1), fq(1, 1)])
            attention(1, 0, [fk(1, 1), fq2(0, 2, 3), None, fk2(0, 2, 3),
                             fv(4), fv(5), fv(6), fv(7)])
            attention(1, 1, [fq2(1, 2, 3), None, fk2(1, 2, 3), None,
                             fv(8), fv(9), fv(10), fv(11)])
            attention(2, 0, [fo(0, 0), None, fo(0, 2), None,
                             fo(0, 4), None, fo(0, 6), None,
                             fv(12), fv(13), fv(14), fv(15)])
            attention(2, 1, [fo(1, 0), None, fo(1, 2), None,
                             fo(1, 4), None, fo(1, 6), None,
                             None, None, None, None])
            attention(3, 0)
            attention(3, 1, [fo(2, 0), None, fo(2, 2), None,
                             fo(2, 4), None, fo(2, 6), None,
                             None, None, None, None, None, None, None, None])
            oproj(3, tail=True)

    nc.compile()
    _NC_CACHE["nc"] = nc
    return nc


def _make_masks():
    # msk[r, p, c] for the 4 diagonal kv offsets r: valid iff p <= c - 128 r
    m = np.zeros((4, P, QT), dtype=np.float16)
    pp = np.arange(P)[:, None]
    cc = np.arange(QT)[None, :]
    for r in range(4):
        m[r] = (pp <= cc - KT * r).astype(np.float16)
    return m


def _in_maps(x, Wq, bq, Wk, bk, Wv, bv, Wo):
    scale = np.float32(1.0 / np.sqrt(HD))
    masks = _make_masks()
    maps = []
    for core in range(NCORES):
        b, g = divmod(core, GROUPS)
        csl = slice(g * CD, (g + 1) * CD)
        maps.append({
            "xt": np.ascontiguousarray(x[b].T).astype(np.float16),
            "wq": np.ascontiguousarray(Wq[:, csl] * scale).astype(np.float16),
            "wk": np.ascontiguousarray(Wk[:, csl]).astype(np.float16),
            "wv": np.ascontiguousarray(Wv[:, csl]).astype(np.float16),
            "wo": np.ascontiguousarray(Wo[csl, :]).astype(np.float16),
            "bq": np.ascontiguousarray(bq[csl] * scale).astype(np.float32),
            "bk": np.ascontiguousarray(bk[csl]).astype(np.float32),
            "bv": np.ascontiguousarray(bv[csl]).reshape(HPG, HD).astype(np.float32),
            "msk": masks,
        })
    return maps


def kernel_with_results(x, Wq, bq, Wk, bk, Wv, bv, Wo, bo, trace=False):
    nc = _build_nc()
    maps = _in_maps(x, Wq, bq, Wk, bk, Wv, bv, Wo)
    kwargs = {}
    if trace:
        kwargs = dict(trace=True, trace_cores=[0])
    res = run_bass_kernel_spmd(nc, maps, core_ids=list(range(NCORES)), **kwargs)
    out = np.zeros((B, S, D), dtype=np.float32)
    for b in range(B):
        acc = np.zeros((D, S), dtype=np.float32)
        for g in range(GROUPS):
            acc += res.results[b * GROUPS + g]["ot"].astype(np.float32)
        out[b] = acc.T + np.asarray(bo, dtype=np.float32)[None, :]
    return out, res


def kernel(x, Wq, bq, Wk, bk, Wv, bv, Wo, bo):
    out, _ = kernel_with_results(x, Wq, bq, Wk, bk, Wv, bv, Wo, bo, trace=False)
    return out
